# revision 4
# baseline (speedup 1.0000x reference)
"""Trainium2 Bass kernel for a bidirectional multi-head LSTM block.

Model (B=128, T=197, D=768, 12 heads x 64 hid):
    y   = x @ pre_w.T + pre_b
    hf  = LSTM_fwd(y)   (block-diagonal per-head gate weights)
    hr  = LSTM_rev(y)
    out = concat(hf, hr) @ proj_w.T + proj_b

Sharding over 8 NeuronCores: (4 head-groups of 3 heads) x (2 batch
halves of 64). Each core runs BOTH directions for its heads/batch —
two independent recurrence chains that pipeline against each other and
hide the serial per-step latency.

Launch 1 (per core): pre-projection slice y_g = x_bh @ pre_w_g.T
(K=768 matmul) emitted from both ends of the time axis (fwd consumes
y from t=0 up, rev from t=T-1 down), interleaved with the 197-step
recurrence.

Recurrence trick: all four gates use tanh only —
    sigmoid(x) = (tanh(x/2) + 1) / 2
with the 1/2 input scales folded into the gate weights host-side, so a
SINGLE ACT instruction evaluates tanh over the whole [128, 384] gate
block per direction per step.  The (t+1)/2 corrections are fused into
scalar_tensor_tensor DVE ops via the rescaled states
    s = 2c,  hdev = 2h:
    q  = (tf + 1) * s_prev
    r  = (ti + 1) * tg
    s  = 0.5*q + r
    tc = tanh(0.5 * s)          (ACT, input scale)
    hdev = (to + 1) * tc
hdev = 2h is written out; the factor 1/2 is folded into proj_w in
launch 2.  Gate chunk layout: chunk0 rows = [f|i], chunk1 rows = [g|o].
Weight folds: W rows f,i,o x0.5 (sigmoid), all rows x0.5 on the W_hh
side for the hdev=2h input (f,i,o: 0.25, g: 0.5).  Bias rows f,i,o
x0.5.  Bias enters via a constant-ones row appended to the h state
(lhsT of the W_hh matmul has 65 rows).

Launch 2 (per core): batch-slice output projection
out_slice.T = proj_w_aug @ lstm_out_slice_aug  (K=1537 padded to 1664,
ones/bias row folds in proj_b; proj_w scaled by 0.5 for hdev).
"""

import os
from contextlib import ExitStack

import numpy as np

import concourse.bass as bass
import concourse.tile as tile
from concourse import bacc, mybir
from concourse.bass_utils import run_bass_kernel_spmd
from concourse.kernels.tile_matmul import matmul_tile_kernel

B, T, D = 128, 197, 768
HEADS, HPH = 12, 64
H = HEADS * HPH  # 768
NCORES = 8
NG = 4                 # head groups
HPG = HEADS // NG      # 3 heads per group
GD = HPG * HPH         # 192 pre-proj cols per group
BC = B // 2            # 64 batch per core
F = HPG * BC           # 192: free dim of one gate chunk (3 heads x 64)
F32 = mybir.dt.float32
ADD = mybir.AluOpType.add
MULT = mybir.AluOpType.mult
TANH = mybir.ActivationFunctionType.Tanh

LAST_RESULTS = []      # stash of BassKernelResults for test harnesses


# --------------------------------------------------------------------------
# Program 1: pre-projection slice + bidirectional LSTM recurrence
# --------------------------------------------------------------------------
def build_lstm_program(t_steps=T):
    TBc = t_steps * BC
    NKT = D // 128         # 6 k-tiles of the pre-projection
    NTILE = 512
    n_ntiles = (TBc + NTILE - 1) // NTILE
    steps_per_ntile = NTILE // BC   # 8

    nc = bacc.Bacc("TRN2", target_bir_lowering=False, debug=False)

    xT = nc.dram_tensor("xT", [D, TBc], F32, kind="ExternalInput")
    preWT = nc.dram_tensor("preWT", [D, GD], F32, kind="ExternalInput")
    # lhsT blocks per (dir d, head j, chunk k) at col 128*((d*HPG+j)*2+k)
    whh = nc.dram_tensor("whh", [HPH + 1, 2 * HPG * 2 * 128], F32,
                         kind="ExternalInput")
    wih = nc.dram_tensor("wih", [HPH, 2 * HPG * 2 * 128], F32,
                         kind="ExternalInput")
    yT = nc.dram_tensor("yT", [GD, TBc], F32)
    # hdev outputs: [r, head j, dir, t, b]
    hsT = nc.dram_tensor("hsT", [HPH, HPG, 2, t_steps, BC], F32,
                         kind="ExternalOutput")

    xT_k = xT.rearrange("(k p) n -> p k n", p=128)            # [128, 6, TBc]
    yT_step = yT.rearrange("(h r) (t b) -> r h t b", h=HPG, b=BC)

    with tile.TileContext(nc) as tc, ExitStack() as ctx:
        # ---------------- pools ----------------
        const = ctx.enter_context(tc.tile_pool(name="const", bufs=1))
        xk_pool = ctx.enter_context(tc.tile_pool(name="xk", bufs=3))
        pa_ps = ctx.enter_context(tc.tile_pool(name="pa_ps", bufs=2, space="PSUM"))
        yout = ctx.enter_context(tc.tile_pool(name="yout", bufs=3))
        ypool = ctx.enter_context(tc.tile_pool(name="ypool", bufs=6))
        rec_ps = ctx.enter_context(tc.tile_pool(name="rec_ps", bufs=2, space="PSUM"))
        gpool = ctx.enter_context(tc.tile_pool(name="gpool", bufs=3))
        state = ctx.enter_context(tc.tile_pool(name="state", bufs=1))

        # ---------------- constants / state ----------------
        pw_sb = const.tile([128, NKT * GD], F32, tag="pw", name="pw_sb")
        pw_sb3 = pw_sb.rearrange("p (k m) -> p k m", k=NKT)
        nc.sync.dma_start(pw_sb3[:], preWT.rearrange("(k p) m -> p k m", p=128))

        whh_sb = const.tile([HPH + 1, 2 * HPG * 2 * 128], F32, tag="whh",
                            name="whh_sb")
        nc.sync.dma_start(whh_sb[:], whh[:])
        wih_sb = const.tile([HPH, 2 * HPG * 2 * 128], F32, tag="wih",
                            name="wih_sb")
        nc.sync.dma_start(wih_sb[:], wih[:])

        h_state = []
        s_state = []
        for d in range(2):
            h_d = state.tile([HPH + 1, F], F32, tag=f"h{d}", name=f"h_state{d}")
            s_d = state.tile([HPH, F], F32, tag=f"s{d}", name=f"s_state{d}")
            nc.vector.memset(h_d[0:HPH, :], 0.0)
            nc.vector.memset(h_d[HPH : HPH + 1, :], 1.0)   # bias row
            nc.vector.memset(s_d[:], 0.0)
            h_state.append(h_d)
            s_state.append(s_d)

        # ---------------- phase-A emission ----------------
        def emit_ntile(ni):
            n0 = ni * NTILE
            nsz = min(NTILE, TBc - n0)
            xk = xk_pool.tile([128, NKT * NTILE], F32, tag="xk", name="xk")
            xk3 = xk.rearrange("p (k n) -> p k n", k=NKT)
            nc.sync.dma_start(xk3[:, :, 0:nsz], xT_k[:, :, n0 : n0 + nsz])
            psA = pa_ps.tile([128, NTILE], F32, tag="psA", name="psA")
            psB = pa_ps.tile([64, NTILE], F32, tag="psB", name="psB")
            for k in range(NKT):
                nc.tensor.matmul(
                    psA[:, 0:nsz], pw_sb3[:, k, 0:128], xk3[:, k, 0:nsz],
                    start=(k == 0), stop=(k == NKT - 1),
                )
            for k in range(NKT):
                nc.tensor.matmul(
                    psB[:, 0:nsz], pw_sb3[:, k, 128:GD], xk3[:, k, 0:nsz],
                    start=(k == 0), stop=(k == NKT - 1),
                )
            ya = yout.tile([128, NTILE], F32, tag="ya", name="ya")
            yb = yout.tile([64, NTILE], F32, tag="yb", name="yb")
            nc.scalar.copy(ya[:, 0:nsz], psA[:, 0:nsz])
            nc.vector.tensor_copy(yb[:, 0:nsz], psB[:, 0:nsz])
            nc.sync.dma_start(yT[0:128, n0 : n0 + nsz], ya[:, 0:nsz])
            nc.sync.dma_start(yT[128:GD, n0 : n0 + nsz], yb[:, 0:nsz])

        front = 0            # next front tile to emit
        back = n_ntiles - 1  # next back tile to emit

        def ensure_tiles(t):
            nonlocal front, back
            want_front = min(n_ntiles - 1, t // steps_per_ntile + 2)
            lo = ((t_steps - 1 - t) * BC) // NTILE
            want_back = max(0, lo - 2)
            while front <= back and (front <= want_front or back >= want_back):
                if front <= want_front:
                    emit_ntile(front)
                    front += 1
                if front > back:
                    break
                if back >= want_back:
                    emit_ntile(back)
                    back -= 1

        # ---------------- one recurrence step of one direction ----------
        def rec_step(d, t):
            t_eff = t if d == 0 else t_steps - 1 - t
            dma_eng = nc.sync if d == 0 else nc.gpsimd

            y_t = ypool.tile([HPH, F], F32, tag=f"y{d}", name=f"y{d}")
            y_t3 = y_t.rearrange("p (h b) -> p h b", h=HPG)
            dma_eng.dma_start(y_t3[:], yT_step[:, :, t_eff, :])

            ps = rec_ps.tile([128, 2 * F], F32, tag=f"ps{d}", name=f"ps{d}")
            # chunk k at cols [k*F : (k+1)*F]; head j at cols k*F + j*BC
            # chunk0 rows = [f|i], chunk1 rows = [g|o]
            for j in range(HPG):
                for k in range(2):
                    o0 = k * F + j * BC
                    blk = 128 * ((d * HPG + j) * 2 + k)
                    nc.tensor.matmul(
                        ps[:, o0 : o0 + BC],
                        wih_sb[:, blk : blk + 128],
                        y_t[:, j * BC : (j + 1) * BC],
                        start=True, stop=False,
                    )
                    nc.tensor.matmul(
                        ps[:, o0 : o0 + BC],
                        whh_sb[:, blk : blk + 128],
                        h_state[d][:, j * BC : (j + 1) * BC],
                        start=False, stop=True,
                    )

            gt = gpool.tile([128, 2 * F], F32, tag=f"g{d}", name=f"g{d}")
            nc.scalar.activation(gt[:], ps[:], TANH)
            # gt cols 0:F rows = [tf|ti]; cols F:2F rows = [to|tg]
            # (partition-base rule: both-SBUF inputs must share base, so
            #  the pairs (ti,tg) sit at base 64 and (to,tc) at base 0)

            q = gpool.tile([64, F], F32, tag=f"q{d}", name=f"q{d}")
            r = gpool.tile([64, F], F32, tag=f"r{d}", name=f"r{d}")
            nc.vector.scalar_tensor_tensor(
                q[:], gt[0:64, 0:F], 1.0, s_state[d][:], ADD, MULT
            )
            nc.vector.scalar_tensor_tensor(
                r[:], gt[64:128, 0:F], 1.0, gt[64:128, F : 2 * F], ADD, MULT
            )
            nc.vector.scalar_tensor_tensor(
                s_state[d][:], q[:], 0.5, r[:], MULT, ADD
            )
            tc_t = gpool.tile([64, F], F32, tag=f"tc{d}", name=f"tc{d}")
            nc.scalar.activation(tc_t[:], s_state[d][:], TANH, scale=0.5)
            nc.vector.scalar_tensor_tensor(
                h_state[d][0:HPH, :], gt[0:64, F : 2 * F], 1.0, tc_t[:],
                ADD, MULT,
            )

            h3 = h_state[d][0:HPH, :].rearrange("p (h b) -> p h b", h=HPG)
            dma_eng.dma_start(hsT[:, :, d, t_eff, :], h3[:])

        # ---------------- time loop ----------------
        for t in range(t_steps):
            ensure_tiles(t)
            rec_step(0, t)
            rec_step(1, t)
        while front <= back:
            emit_ntile(front)
            front += 1

    nc.compile()
    return nc


# --------------------------------------------------------------------------
# Program 2: output projection for a batch slice
# --------------------------------------------------------------------------
KPAD = 13 * 128            # 1664 (>= 2H+1 bias row)


def build_proj_program():
    Bc = B // NCORES       # 16
    TBc2 = T * Bc          # 3152
    nc = bacc.Bacc("TRN2", target_bir_lowering=False, debug=False)
    lstmT = nc.dram_tensor("lstmT", [KPAD, TBc2], F32, kind="ExternalInput")
    projWT = nc.dram_tensor("projWT", [KPAD, D], F32, kind="ExternalInput")
    outT = nc.dram_tensor("outT", [D, TBc2], F32, kind="ExternalOutput")
    with tile.TileContext(nc) as tc:
        matmul_tile_kernel(tc, projWT[:], lstmT[:], outT[:])
    nc.compile()
    return nc


# --------------------------------------------------------------------------
# Host-side weight prep
# --------------------------------------------------------------------------
def _head_rows(h):
    """Gate rows of head h in the [4H, *] weights, reordered to the device
    chunk layout: chunk0 = [f|i], chunk1 = [o|g]  (pytorch order i,f,g,o)."""
    i = np.arange(0 * H + h * HPH, 0 * H + (h + 1) * HPH)
    f = np.arange(1 * H + h * HPH, 1 * H + (h + 1) * HPH)
    g = np.arange(2 * H + h * HPH, 2 * H + (h + 1) * HPH)
    o = np.arange(3 * H + h * HPH, 3 * H + (h + 1) * HPH)
    return np.concatenate([f, i, o, g])       # 256 rows


# per-row folds over the 256 reordered rows [f,i,o,g]
_SIG = np.concatenate([np.full(64, 0.5), np.full(64, 0.5),
                       np.full(64, 0.5), np.full(64, 1.0)]).astype(np.float32)


def _prep_weights(pre_w, pre_b, dirs, heads):
    """preWT [768,GD]; whh [65, 2*6*128]; wih [64, 2*6*128] for one core.
    dirs = [(w_ih_f, w_hh_f, b_ih_f, b_hh_f), (reverse...)]."""
    preWT = np.concatenate(
        [pre_w[h * HPH : (h + 1) * HPH, :] for h in heads], axis=0
    ).T.copy()
    whh = np.zeros((HPH + 1, 2 * HPG * 2 * 128), np.float32)
    wih = np.zeros((HPH, 2 * HPG * 2 * 128), np.float32)
    for d, (w_ih, w_hh, b_ih, b_hh) in enumerate(dirs):
        for j, h in enumerate(heads):
            rows = _head_rows(h)
            cols = np.arange(h * HPH, (h + 1) * HPH)
            Wih = w_ih[np.ix_(rows, cols)] * _SIG[:, None]          # [256,64]
            Whh = w_hh[np.ix_(rows, cols)] * (_SIG * 0.5)[:, None]  # hdev=2h
            bias = (Wih @ pre_b[cols]) + (b_ih[rows] + b_hh[rows]) * _SIG
            for k in range(2):
                blk = 128 * ((d * HPG + j) * 2 + k)
                sl = slice(k * 128, (k + 1) * 128)
                whh[0:HPH, blk : blk + 128] = Whh[sl, :].T
                whh[HPH, blk : blk + 128] = bias[sl]
                wih[:, blk : blk + 128] = Wih[sl, :].T
    return preWT, whh, wih


# --------------------------------------------------------------------------
# Main entry
# --------------------------------------------------------------------------
def kernel(
    x,
    pre_w,
    pre_b,
    w_ih_f,
    w_hh_f,
    b_ih_f,
    b_hh_f,
    w_ih_r,
    w_hh_r,
    b_ih_r,
    b_hh_r,
    proj_w,
    proj_b,
):
    trace = bool(os.environ.get("KERNEL_TRACE"))
    LAST_RESULTS.clear()
    core_ids = list(range(NCORES))
    dirs = [
        (w_ih_f, w_hh_f, b_ih_f, b_hh_f),
        (w_ih_r, w_hh_r, b_ih_r, b_hh_r),
    ]

    # ---- launch 1 inputs: core c = (group g, batch-half bh)
    xT_bh = [
        np.ascontiguousarray(
            x[bh * BC : (bh + 1) * BC].transpose(2, 1, 0).reshape(D, T * BC)
        )
        for bh in range(2)
    ]
    in_maps1 = []
    for c in core_ids:
        g, bh = divmod(c, 2)
        heads = [g * HPG + j for j in range(HPG)]
        preWT, whh, wih = _prep_weights(pre_w, pre_b, dirs, heads)
        in_maps1.append(
            {"xT": xT_bh[bh], "preWT": preWT, "whh": whh, "wih": wih}
        )

    nc1 = build_lstm_program()
    res1 = run_bass_kernel_spmd(nc1, in_maps1, core_ids, trace=trace)
    LAST_RESULTS.append(res1)

    # ---- assemble lstm_out rows [1536, B, T] (values are hdev = 2h)
    lstm_rows = np.empty((2 * H, B, T), np.float32)
    for c in core_ids:
        g, bh = divmod(c, 2)
        hs = res1.results[c]["hsT"]          # [64, 3, 2, T, 64]
        for d in range(2):
            arr = hs[:, :, d].transpose(1, 0, 3, 2)   # [3, 64, BC, T]
            r0 = d * H + g * HPG * HPH
            lstm_rows[r0 : r0 + GD, bh * BC : (bh + 1) * BC] = arr.reshape(
                GD, BC, T
            )

    # ---- launch 2 (proj_w scaled 0.5 to undo hdev = 2h)
    Bc = B // NCORES
    TBc2 = T * Bc
    projWT = np.zeros((KPAD, D), np.float32)
    projWT[: 2 * H] = proj_w.T * 0.5
    projWT[2 * H] = proj_b
    in_maps2 = []
    for c in core_ids:
        kxn = np.zeros((KPAD, TBc2), np.float32)
        kxn[: 2 * H] = lstm_rows[:, c * Bc : (c + 1) * Bc, :].reshape(2 * H, TBc2)
        kxn[2 * H] = 1.0
        in_maps2.append({"lstmT": kxn, "projWT": projWT})

    nc2 = build_proj_program()
    res2 = run_bass_kernel_spmd(nc2, in_maps2, core_ids, trace=trace)
    LAST_RESULTS.append(res2)

    out = np.empty((B, T, D), np.float32)
    for c in core_ids:
        outT = res2.results[c]["outT"]       # [768, 3152]
        out[c * Bc : (c + 1) * Bc] = outT.reshape(D, Bc, T).transpose(1, 2, 0)
    return out
